# revision 3
# baseline (speedup 1.0000x reference)
"""dX-privacy embedding snap (argmax over vocab of noisy-embedding scores)
for Trainium2, 8 NeuronCores — fp8 DoubleRow edition.

Distribution: vocab-sharded. Core c owns rows [c*4000, (c+1)*4000) of the
embedding table; every core scores all 8192 tokens against its shard.

Device (per core): x^T and E^T quantized to fp8e4 (host-side); matmuls run in
MatmulPerfMode.DoubleRow (256-deep contraction per instruction, 2x fp16
throughput). The full E^T shard (16 MB fp8) stays resident in SBUF. For each
128-token tile, 8 PSUM banks hold [128, 500] f32 scores; each bank carries TWO
250-wide accumulation regions (DoubleRow moving limit is 512), the second
relying on the bank-wide pending-zero armed by the first region's start=True.
Consecutive matmuls alternate stationary tiles (measured faster than reuse).
ACT engine bounces PSUM->SBUF as bf16; DVE max/max_index emit per-token top-8
values + local indices over the core's 4000 vocab rows.

Host: merge 8 cores x top-8 = 64 candidates, take top-16 by fp8 score (the
true winner's fp8 rank within its core was measured <= 2 on this data; score
noise sigma ~= 2.5 vs mean top-gap ~15), rescore the 16 exactly in fp32 with
jnp (reference-identical arithmetic), and fully rescore any token whose
candidate margin < 0.02 against the whole vocab. Output = embed_table[winner].
"""

import sys, os, time

sys.path.insert(0, "/opt/trn_rl_repo")
import numpy as np
import ml_dtypes

import bass_rust
import concourse.bass as bass
import concourse.mybir as mybir
from concourse import tile


f32 = mybir.dt.float32
f16 = mybir.dt.float16
bf16 = mybir.dt.bfloat16
f8 = mybir.dt.float8e4
u32 = mybir.dt.uint32
DR = mybir.MatmulPerfMode.DoubleRow

B, S, D, V = 4, 2048, 4096, 32000
T = B * S  # 8192 tokens
N_CORES = 8
VSH = V // N_CORES  # 4000 vocab rows per core
KP = D // 256  # 16 k-pair tiles (256-deep DoubleRow contraction each)
NBANK = 8  # psum banks, each [128, 500] f32 = one 2KB bank
NV = 250  # cols per matmul (moving 2*250=500 <= 512)
NTB = 8  # t blocks
TB = T // NTB  # 1024 tokens per t block
NTT = TB // 128  # 8 token tiles per block
C_RESCORE = 16  # exact-rescore candidates per token
FLAG_THETA = 0.02  # full-vocab rescore margin

BANK_SHARE = True  # two 250-wide regions per psum bank (pending-zero trick)

_mwfix_ctr = [0]


def _legalize_multiwaits(nc, max_waits=1):
    """walrus encodes at most one sem wait per instruction; split multi-wait
    instructions by inserting single-wait NOPs before them (same engine)."""
    for fn in nc.m.functions:
        for bb in fn.blocks:
            insts = list(bb.instructions)
            out = []
            changed = False
            for inst in insts:
                si = inst.sync_info
                ow = list(si.on_wait) if si is not None and si.on_wait else []
                if len(ow) > max_waits:
                    for wentry in ow[:-max_waits]:
                        _mwfix_ctr[0] += 1
                        nop = mybir.InstNoOp(
                            name=f"mwfix-{_mwfix_ctr[0]}", ins=[], outs=[]
                        )
                        nop.engine = inst.engine
                        nop.sync_info = bass_rust.SyncInfo(
                            on_wait=[wentry], on_update=[]
                        )
                        out.append(nop)
                    si.on_wait = ow[-max_waits:]
                    changed = True
                out.append(inst)
            if changed:
                bb.instructions = out


def _build_nc():
    nc = bass.Bass()
    # xt: x^T in fp8, [tb][k'][128 p][2 i][1024 tok]; d = k'*256 + i*128 + p
    xt_in = nc.declare_dram_parameter("xt", [NTB, KP, 128, 2, TB], f8, isOutput=False)
    # et: E-shard^T in fp8, [k'][j 500-block][128 p][2 i][500 v]
    et_in = nc.declare_dram_parameter("et", [KP, 8, 128, 2, 2 * NV], f8, isOutput=False)
    out_val = nc.declare_dram_parameter("val8", [NTB * NTT, 128, 8], bf16, isOutput=True)
    out_idx = nc.declare_dram_parameter("idx8", [NTB * NTT, 128, 8], u32, isOutput=True)

    with tile.TileContext(nc) as tc:
        with (
            tc.tile_pool(name="et", bufs=1) as etp,
            tc.tile_pool(name="xt", bufs=2) as xtp,
            tc.tile_pool(name="bnc", bufs=2) as bncp,
            tc.tile_pool(name="o8", bufs=2) as o8p,
            tc.tile_pool(name="ps", bufs=1, space="PSUM") as ps,
        ):
            # resident E^T shard: 16 k' x 8 j tiles of [128, 2, 500] fp8
            et_tiles = {}
            for kp in range(KP):
                for j in range(8):
                    t = etp.tile(
                        [128, 2, 2 * NV], f8, tag=f"et{kp}_{j}", name=f"et_{kp}_{j}"
                    )
                    nc.sync.dma_start(t[:], et_in[kp, j])
                    et_tiles[(kp, j)] = t

            for tb in range(NTB):
                # double-buffered x^T tiles for this token block
                xt_tiles = []
                for kp in range(KP):
                    t = xtp.tile([128, 2, TB], f8, tag=f"xt{kp}", name=f"xt_{tb}_{kp}")
                    nc.scalar.dma_start(t[:], xt_in[tb, kp])
                    xt_tiles.append(t)

                for tt in range(NTT):
                    bounce = bncp.tile(
                        [128, NBANK * 2 * NV], bf16, tag="bnc", name=f"bn_{tb}_{tt}"
                    )
                    lhs = [
                        xt_tiles[k][:, :, tt * 128 : (tt + 1) * 128]
                        for k in range(KP)
                    ]
                    # two passes of 4 banks x 500-wide matmuls; the pass's
                    # psum->bounce copies overlap the next pass's compute.
                    for half in range(2):
                        psums = [
                            ps.tile(
                                [128, 2 * NV], f32,
                                tag=f"ps{half * 4 + b}",
                                name=f"ps_{tb}_{tt}_{half}_{b}",
                            )
                            for b in range(4)
                        ]
                        # consecutive matmuls alternate stationaries; bank b
                        # sees both parities of each k' pair across j and kk
                        for kk in range(KP // 2):
                            for j in range(8):
                                b = j % 4
                                par = (j + j // 4) % 2
                                k = 2 * kk + par
                                nc.tensor.matmul(
                                    psums[b][:],
                                    lhs[k],
                                    et_tiles[(k, half * 4 + b)][:],
                                    start=(kk == 0 and j < 4),
                                    stop=(kk == KP // 2 - 1 and j >= 4),
                                    perf_mode=DR,
                                )
                        for b in range(4):
                            gb = half * 4 + b
                            nc.scalar.copy(
                                out=bounce[:, gb * 2 * NV : (gb + 1) * 2 * NV],
                                in_=psums[b][:],
                            )
                    val8 = o8p.tile([128, 8], bf16, tag="val8", name=f"v8_{tb}_{tt}")
                    idx8 = o8p.tile([128, 8], u32, tag="idx8", name=f"i8_{tb}_{tt}")
                    nc.vector.max(out=val8[:], in_=bounce[:])
                    nc.vector.max_index(out=idx8[:], in_max=val8[:], in_values=bounce[:])
                    nc.sync.dma_start(out_val[tb * NTT + tt], val8[:])
                    nc.sync.dma_start(out_idx[tb * NTT + tt], idx8[:])
    _legalize_multiwaits(nc)
    return nc


_RUNNER = None
LAST_TIMES = None


def _get_runner():
    global _RUNNER
    if _RUNNER is not None:
        return _RUNNER
    import jax
    from jax.sharding import Mesh, PartitionSpec, NamedSharding
    from jax.experimental.shard_map import shard_map
    from concourse.bass2jax import (
        _bass_exec_p,
        install_neuronx_cc_hook,
        partition_id_tensor,
    )

    nc = _build_nc()
    install_neuronx_cc_hook()
    partition_name = nc.partition_id_tensor.name if nc.partition_id_tensor else None

    in_names, out_names, out_avals, zero_outs = [], [], [], []
    for alloc in nc.m.functions[0].allocations:
        if not isinstance(alloc, mybir.MemoryLocationSet):
            continue
        name = alloc.memorylocations[0].name
        if alloc.kind == "ExternalInput":
            if name != partition_name:
                in_names.append(name)
        elif alloc.kind == "ExternalOutput":
            shape, dt = alloc.tensor_shape, mybir.dt.np(alloc.dtype)
            out_names.append(name)
            out_avals.append(jax.core.ShapedArray(shape, dt))
            zero_outs.append(np.zeros(shape, dt))

    n_params = len(in_names)
    all_in_names = list(in_names) + list(out_names)
    if partition_name is not None:
        all_in_names.append(partition_name)

    def _body(*args):
        operands = list(args)
        if partition_name is not None:
            operands.append(partition_id_tensor())
        outs = _bass_exec_p.bind(
            *operands,
            out_avals=tuple(out_avals),
            in_names=tuple(all_in_names),
            out_names=tuple(out_names),
            lowering_input_output_aliases=(),
            sim_require_finite=True,
            sim_require_nnan=True,
            nc=nc,
        )
        return tuple(outs)

    devices = jax.devices()[:N_CORES]
    mesh = Mesh(np.asarray(devices), ("core",))
    in_specs = (PartitionSpec("core"),) * (n_params + len(out_names))
    out_specs = (PartitionSpec("core"),) * len(out_names)
    fn = jax.jit(
        shard_map(
            _body, mesh=mesh, in_specs=in_specs, out_specs=out_specs, check_rep=False
        ),
        keep_unused=True,
    )

    def run(in_maps, n_iters=1):
        global LAST_TIMES
        args = []
        for name in in_names:
            shards = [
                jax.device_put(np.ascontiguousarray(in_maps[c][name]), devices[c])
                for c in range(N_CORES)
            ]
            per_shape = shards[0].shape
            gshape = (N_CORES * per_shape[0],) + tuple(per_shape[1:])
            args.append(
                jax.make_array_from_single_device_arrays(
                    gshape, NamedSharding(mesh, PartitionSpec("core")), shards
                )
            )
        zargs = []
        for z in zero_outs:
            shards = [jax.device_put(z, d) for d in devices]
            gshape = (N_CORES * z.shape[0],) + tuple(z.shape[1:])
            zargs.append(
                jax.make_array_from_single_device_arrays(
                    gshape, NamedSharding(mesh, PartitionSpec("core")), shards
                )
            )
        out = fn(*args, *zargs)
        jax.block_until_ready(out)
        globals()["_FN"] = fn
        globals()["_ARGS"] = (args, zargs)
        times = []
        for _ in range(n_iters - 1):
            t0 = time.perf_counter()
            out = fn(*args, *zargs)
            jax.block_until_ready(out)
            times.append(time.perf_counter() - t0)
        LAST_TIMES = times
        results = []
        for c in range(N_CORES):
            m = {}
            for i, name in enumerate(out_names):
                ga = np.asarray(out[i]).reshape((N_CORES,) + out_avals[i].shape)
                m[name] = ga[c]
            results.append(m)
        return results

    _RUNNER = run
    return run


def measure_exec_ns(chains=(20, 120), tries=3):
    import jax

    fn = globals().get("_FN")
    args, zargs = globals().get("_ARGS")
    best = None
    for _ in range(tries):
        ts = []
        for n in chains:
            o = fn(*args, *zargs)
            jax.block_until_ready(o)
            t0 = time.perf_counter()
            for _ in range(n):
                o = fn(*args, *zargs)
            jax.block_until_ready(o)
            ts.append(time.perf_counter() - t0)
        per = (ts[1] - ts[0]) / (chains[1] - chains[0])
        best = per if best is None else min(best, per)
    return best * 1e9


def kernel(inputs_embeds, embed_table, noise):
    verbose = os.environ.get("KERNEL_VERBOSE")
    _t = [time.time()]

    def _lap(msg):
        if verbose:
            t = time.time()
            print(f"[kernel] {msg}: {t - _t[0]:.1f}s", flush=True)
            _t[0] = t

    inputs_embeds = np.asarray(inputs_embeds)
    embed_table = np.asarray(embed_table)
    noise = np.asarray(noise)

    # host prep: quantize to fp8 and lay out for DoubleRow
    x = (inputs_embeds + noise).reshape(T, D).astype(np.float32)
    x8 = x.astype(ml_dtypes.float8_e4m3)
    # [tb, tok, k', i, p] -> [tb, k', p, i, tok]
    xt8 = np.ascontiguousarray(
        x8.reshape(NTB, TB, KP, 2, 128).transpose(0, 2, 4, 3, 1)
    )
    in_maps = []
    for c in range(N_CORES):
        sh8 = embed_table[c * VSH : (c + 1) * VSH].astype(ml_dtypes.float8_e4m3)
        # [j, v, k', i, p] -> [k', j, p, i, v]
        et8 = np.ascontiguousarray(
            sh8.reshape(8, 2 * NV, KP, 2, 128).transpose(2, 0, 4, 3, 1)
        )
        in_maps.append({"xt": xt8, "et": et8})
    _lap("host prep")

    run = _get_runner()
    _lap("compile/runner")
    n_iters = int(os.environ.get("KERNEL_TIME_ITERS", "1"))
    results = run(in_maps, n_iters=n_iters)
    _lap("stage+run")

    # host merge: 8 cores x top-8 -> top-C_RESCORE by fp8 score
    cand_vals = np.empty((T, N_CORES * 8), dtype=np.float32)
    cand_idx = np.empty((T, N_CORES * 8), dtype=np.int64)
    for c in range(N_CORES):
        v8 = results[c]["val8"].astype(np.float32).reshape(T, 8)
        i8 = results[c]["idx8"].astype(np.int64).reshape(T, 8)
        cand_vals[:, c * 8 : (c + 1) * 8] = v8
        cand_idx[:, c * 8 : (c + 1) * 8] = c * VSH + i8

    top = np.argpartition(-cand_vals, C_RESCORE - 1, axis=1)[:, :C_RESCORE]
    rows = np.arange(T)[:, None]
    cidx = cand_idx[rows, top]  # [T, C]

    # exact fp32 rescore of the C candidates (reference-identical arithmetic)
    import jax as _jax
    import jax.numpy as jnp

    with _jax.default_device(_jax.devices("cpu")[0]):
        win_idx = np.empty(T, dtype=np.int64)
        margin = np.empty(T, dtype=np.float32)
        CH = 2048
        for c0 in range(0, T, CH):
            sl = slice(c0, c0 + CH)
            ec = jnp.asarray(embed_table[cidx[sl]])  # [CH, C, D]
            xs = jnp.asarray(x[sl])  # [CH, D]
            s = np.asarray(jnp.einsum("tcd,td->tc", ec, xs))
            order = np.argsort(-s, axis=1)
            bi = order[:, 0]
            r = np.arange(s.shape[0])
            win_idx[sl] = cidx[sl][r, bi]
            margin[sl] = s[r, bi] - s[r, order[:, 1]]

        # safety net: full-vocab rescore of low-margin tokens
        flagged = np.where(margin < FLAG_THETA)[0]
        if flagged.size:
            s = jnp.einsum("td,vd->tv", jnp.asarray(x[flagged]), jnp.asarray(embed_table))
            win_idx[flagged] = np.asarray(jnp.argmax(s, axis=-1))

    _lap(f"merge+rescore ({flagged.size} flagged)")
    out = embed_table[win_idx].reshape(B, S, D)
    _lap("gather")
    return out


# revision 4
# speedup vs baseline: 1.2414x; 1.2414x over previous
"""dX-privacy embedding snap (argmax over vocab of noisy-embedding scores)
for Trainium2, 8 NeuronCores — fp8 DoubleRow edition.

Distribution: vocab-sharded. Core c owns rows [c*4000, (c+1)*4000) of the
embedding table; every core scores all 8192 tokens against its shard.

Device (per core): x^T and E^T quantized to fp8e4 (host-side); matmuls run in
MatmulPerfMode.DoubleRow (256-deep contraction per instruction, 2x fp16
throughput; 500-wide moving/out measured fastest: ~40 fixed cycles per
instruction, so widest legal wins). The full E^T shard (16 MB fp8) stays
resident in SBUF. Per 128-token tile, scores accumulate in two passes of four
[128, 500] f32 PSUM banks; each pass's ACT-engine PSUM->bf16 bounce copies
overlap the other pass's matmuls, so the tensor engine never waits on drains.
DVE max/max_index emit per-token top-8 values + local indices over the core's
4000 vocab rows.

Host: merge 8 cores x top-8 = 64 candidates, take top-16 by fp8 score (the
true winner's fp8 rank within its core was measured <= 2 on this data; score
noise sigma ~= 2.5 vs mean top-gap ~15), rescore the 16 exactly in fp32 with
jnp (reference-identical arithmetic), and fully rescore any token whose
candidate margin < 0.02 against the whole vocab. Output = embed_table[winner].
"""

import sys, os, time

sys.path.insert(0, "/opt/trn_rl_repo")
import numpy as np
import ml_dtypes

import bass_rust
import concourse.bass as bass
import concourse.mybir as mybir
from concourse import tile


f32 = mybir.dt.float32
f16 = mybir.dt.float16
bf16 = mybir.dt.bfloat16
f8 = mybir.dt.float8e4
u32 = mybir.dt.uint32
DR = mybir.MatmulPerfMode.DoubleRow

B, S, D, V = 4, 2048, 4096, 32000
T = B * S  # 8192 tokens
N_CORES = 8
VSH = V // N_CORES  # 4000 vocab rows per core
KP = D // 256  # 16 k-pair tiles (256-deep DoubleRow contraction each)
NBANK = 8  # psum banks, each [128, 500] f32 = one 2KB bank
NV = 250  # half-width: each matmul covers 2*NV=500 cols (one full psum bank)
NTB = 8  # t blocks
TB = T // NTB  # 1024 tokens per t block
NTT = TB // 128  # 8 token tiles per block
C_RESCORE = 16  # exact-rescore candidates per token
FLAG_THETA = 0.02  # full-vocab rescore margin

_mwfix_ctr = [0]


def _legalize_multiwaits(nc, max_waits=1):
    """walrus encodes at most one sem wait per instruction; split multi-wait
    instructions by inserting single-wait NOPs before them (same engine)."""
    for fn in nc.m.functions:
        for bb in fn.blocks:
            insts = list(bb.instructions)
            out = []
            changed = False
            for inst in insts:
                si = inst.sync_info
                ow = list(si.on_wait) if si is not None and si.on_wait else []
                if len(ow) > max_waits:
                    for wentry in ow[:-max_waits]:
                        _mwfix_ctr[0] += 1
                        nop = mybir.InstNoOp(
                            name=f"mwfix-{_mwfix_ctr[0]}", ins=[], outs=[]
                        )
                        nop.engine = inst.engine
                        nop.sync_info = bass_rust.SyncInfo(
                            on_wait=[wentry], on_update=[]
                        )
                        out.append(nop)
                    si.on_wait = ow[-max_waits:]
                    changed = True
                out.append(inst)
            if changed:
                bb.instructions = out


def _build_nc():
    nc = bass.Bass()
    # xt: x^T in fp8, [tb][k'][128 p][2 i][1024 tok]; d = k'*256 + i*128 + p
    xt_in = nc.declare_dram_parameter("xt", [NTB, KP, 128, 2, TB], f8, isOutput=False)
    # et: E-shard^T in fp8, [k'][j 500-block][128 p][2 i][500 v]
    et_in = nc.declare_dram_parameter("et", [KP, 8, 128, 2, 2 * NV], f8, isOutput=False)
    out_val = nc.declare_dram_parameter("val8", [NTB * NTT, 128, 8], bf16, isOutput=True)
    out_idx = nc.declare_dram_parameter("idx8", [NTB * NTT, 128, 8], u32, isOutput=True)

    with tile.TileContext(nc) as tc:
        with (
            tc.tile_pool(name="et", bufs=1) as etp,
            tc.tile_pool(name="xt", bufs=2) as xtp,
            tc.tile_pool(name="bnc", bufs=2) as bncp,
            tc.tile_pool(name="o8", bufs=2) as o8p,
            tc.tile_pool(name="ps", bufs=1, space="PSUM") as ps,
        ):
            # resident E^T shard: 16 k' x 8 j tiles of [128, 2, 500] fp8
            et_tiles = {}
            for kp in range(KP):
                for j in range(8):
                    t = etp.tile(
                        [128, 2, 2 * NV], f8, tag=f"et{kp}_{j}", name=f"et_{kp}_{j}"
                    )
                    nc.sync.dma_start(t[:], et_in[kp, j])
                    et_tiles[(kp, j)] = t

            for tb in range(NTB):
                # double-buffered x^T tiles for this token block
                xt_tiles = []
                for kp in range(KP):
                    t = xtp.tile([128, 2, TB], f8, tag=f"xt{kp}", name=f"xt_{tb}_{kp}")
                    nc.scalar.dma_start(t[:], xt_in[tb, kp])
                    xt_tiles.append(t)

                for tt in range(NTT):
                    bounce = bncp.tile(
                        [128, NBANK * 2 * NV], bf16, tag="bnc", name=f"bn_{tb}_{tt}"
                    )
                    lhs = [
                        xt_tiles[k][:, :, tt * 128 : (tt + 1) * 128]
                        for k in range(KP)
                    ]
                    # two passes of 4 banks x 500-wide matmuls; the pass's
                    # psum->bounce copies overlap the next pass's compute.
                    for half in range(2):
                        psums = [
                            ps.tile(
                                [128, 2 * NV], f32,
                                tag=f"ps{half * 4 + b}",
                                name=f"ps_{tb}_{tt}_{half}_{b}",
                            )
                            for b in range(4)
                        ]
                        # consecutive matmuls alternate stationaries; bank b
                        # sees both parities of each k' pair across j and kk
                        for kk in range(KP // 2):
                            for j in range(8):
                                b = j % 4
                                par = (j + j // 4) % 2
                                k = 2 * kk + par
                                nc.tensor.matmul(
                                    psums[b][:],
                                    lhs[k],
                                    et_tiles[(k, half * 4 + b)][:],
                                    start=(kk == 0 and j < 4),
                                    stop=(kk == KP // 2 - 1 and j >= 4),
                                    perf_mode=DR,
                                )
                        for b in range(4):
                            gb = half * 4 + b
                            nc.scalar.copy(
                                out=bounce[:, gb * 2 * NV : (gb + 1) * 2 * NV],
                                in_=psums[b][:],
                            )
                    val8 = o8p.tile([128, 8], bf16, tag="val8", name=f"v8_{tb}_{tt}")
                    idx8 = o8p.tile([128, 8], u32, tag="idx8", name=f"i8_{tb}_{tt}")
                    nc.vector.max(out=val8[:], in_=bounce[:])
                    nc.vector.max_index(out=idx8[:], in_max=val8[:], in_values=bounce[:])
                    nc.sync.dma_start(out_val[tb * NTT + tt], val8[:])
                    nc.sync.dma_start(out_idx[tb * NTT + tt], idx8[:])
    _legalize_multiwaits(nc)
    return nc


_RUNNER = None
LAST_TIMES = None


def _get_runner():
    global _RUNNER
    if _RUNNER is not None:
        return _RUNNER
    import jax
    from jax.sharding import Mesh, PartitionSpec, NamedSharding
    from jax.experimental.shard_map import shard_map
    from concourse.bass2jax import (
        _bass_exec_p,
        install_neuronx_cc_hook,
        partition_id_tensor,
    )

    nc = _build_nc()
    install_neuronx_cc_hook()
    partition_name = nc.partition_id_tensor.name if nc.partition_id_tensor else None

    in_names, out_names, out_avals, zero_outs = [], [], [], []
    for alloc in nc.m.functions[0].allocations:
        if not isinstance(alloc, mybir.MemoryLocationSet):
            continue
        name = alloc.memorylocations[0].name
        if alloc.kind == "ExternalInput":
            if name != partition_name:
                in_names.append(name)
        elif alloc.kind == "ExternalOutput":
            shape, dt = alloc.tensor_shape, mybir.dt.np(alloc.dtype)
            out_names.append(name)
            out_avals.append(jax.core.ShapedArray(shape, dt))
            zero_outs.append(np.zeros(shape, dt))

    n_params = len(in_names)
    all_in_names = list(in_names) + list(out_names)
    if partition_name is not None:
        all_in_names.append(partition_name)

    def _body(*args):
        operands = list(args)
        if partition_name is not None:
            operands.append(partition_id_tensor())
        outs = _bass_exec_p.bind(
            *operands,
            out_avals=tuple(out_avals),
            in_names=tuple(all_in_names),
            out_names=tuple(out_names),
            lowering_input_output_aliases=(),
            sim_require_finite=True,
            sim_require_nnan=True,
            nc=nc,
        )
        return tuple(outs)

    devices = jax.devices()[:N_CORES]
    mesh = Mesh(np.asarray(devices), ("core",))
    in_specs = (PartitionSpec("core"),) * (n_params + len(out_names))
    out_specs = (PartitionSpec("core"),) * len(out_names)
    fn = jax.jit(
        shard_map(
            _body, mesh=mesh, in_specs=in_specs, out_specs=out_specs, check_rep=False
        ),
        keep_unused=True,
    )

    def run(in_maps, n_iters=1):
        global LAST_TIMES
        args = []
        for name in in_names:
            shards = [
                jax.device_put(np.ascontiguousarray(in_maps[c][name]), devices[c])
                for c in range(N_CORES)
            ]
            per_shape = shards[0].shape
            gshape = (N_CORES * per_shape[0],) + tuple(per_shape[1:])
            args.append(
                jax.make_array_from_single_device_arrays(
                    gshape, NamedSharding(mesh, PartitionSpec("core")), shards
                )
            )
        zargs = []
        for z in zero_outs:
            shards = [jax.device_put(z, d) for d in devices]
            gshape = (N_CORES * z.shape[0],) + tuple(z.shape[1:])
            zargs.append(
                jax.make_array_from_single_device_arrays(
                    gshape, NamedSharding(mesh, PartitionSpec("core")), shards
                )
            )
        out = fn(*args, *zargs)
        jax.block_until_ready(out)
        globals()["_FN"] = fn
        globals()["_ARGS"] = (args, zargs)
        times = []
        for _ in range(n_iters - 1):
            t0 = time.perf_counter()
            out = fn(*args, *zargs)
            jax.block_until_ready(out)
            times.append(time.perf_counter() - t0)
        LAST_TIMES = times
        results = []
        for c in range(N_CORES):
            m = {}
            for i, name in enumerate(out_names):
                ga = np.asarray(out[i]).reshape((N_CORES,) + out_avals[i].shape)
                m[name] = ga[c]
            results.append(m)
        return results

    _RUNNER = run
    return run


def measure_exec_ns(chains=(20, 120), tries=3):
    import jax

    fn = globals().get("_FN")
    args, zargs = globals().get("_ARGS")
    best = None
    for _ in range(tries):
        ts = []
        for n in chains:
            o = fn(*args, *zargs)
            jax.block_until_ready(o)
            t0 = time.perf_counter()
            for _ in range(n):
                o = fn(*args, *zargs)
            jax.block_until_ready(o)
            ts.append(time.perf_counter() - t0)
        per = (ts[1] - ts[0]) / (chains[1] - chains[0])
        best = per if best is None else min(best, per)
    return best * 1e9


def kernel(inputs_embeds, embed_table, noise):
    verbose = os.environ.get("KERNEL_VERBOSE")
    _t = [time.time()]

    def _lap(msg):
        if verbose:
            t = time.time()
            print(f"[kernel] {msg}: {t - _t[0]:.1f}s", flush=True)
            _t[0] = t

    inputs_embeds = np.asarray(inputs_embeds)
    embed_table = np.asarray(embed_table)
    noise = np.asarray(noise)

    # host prep: quantize to fp8 and lay out for DoubleRow
    x = (inputs_embeds + noise).reshape(T, D).astype(np.float32)
    x8 = x.astype(ml_dtypes.float8_e4m3)
    # [tb, tok, k', i, p] -> [tb, k', p, i, tok]
    xt8 = np.ascontiguousarray(
        x8.reshape(NTB, TB, KP, 2, 128).transpose(0, 2, 4, 3, 1)
    )
    in_maps = []
    for c in range(N_CORES):
        sh8 = embed_table[c * VSH : (c + 1) * VSH].astype(ml_dtypes.float8_e4m3)
        # [j, v, k', i, p] -> [k', j, p, i, v]
        et8 = np.ascontiguousarray(
            sh8.reshape(8, 2 * NV, KP, 2, 128).transpose(2, 0, 4, 3, 1)
        )
        in_maps.append({"xt": xt8, "et": et8})
    _lap("host prep")

    run = _get_runner()
    _lap("compile/runner")
    n_iters = int(os.environ.get("KERNEL_TIME_ITERS", "1"))
    results = run(in_maps, n_iters=n_iters)
    _lap("stage+run")

    # host merge: 8 cores x top-8 -> top-C_RESCORE by fp8 score
    cand_vals = np.empty((T, N_CORES * 8), dtype=np.float32)
    cand_idx = np.empty((T, N_CORES * 8), dtype=np.int64)
    for c in range(N_CORES):
        v8 = results[c]["val8"].astype(np.float32).reshape(T, 8)
        i8 = results[c]["idx8"].astype(np.int64).reshape(T, 8)
        cand_vals[:, c * 8 : (c + 1) * 8] = v8
        cand_idx[:, c * 8 : (c + 1) * 8] = c * VSH + i8

    top = np.argpartition(-cand_vals, C_RESCORE - 1, axis=1)[:, :C_RESCORE]
    rows = np.arange(T)[:, None]
    cidx = cand_idx[rows, top]  # [T, C]

    # exact fp32 rescore of the C candidates (reference-identical arithmetic)
    import jax as _jax
    import jax.numpy as jnp

    with _jax.default_device(_jax.devices("cpu")[0]):
        win_idx = np.empty(T, dtype=np.int64)
        margin = np.empty(T, dtype=np.float32)
        CH = 2048
        for c0 in range(0, T, CH):
            sl = slice(c0, c0 + CH)
            ec = jnp.asarray(embed_table[cidx[sl]])  # [CH, C, D]
            xs = jnp.asarray(x[sl])  # [CH, D]
            s = np.asarray(jnp.einsum("tcd,td->tc", ec, xs))
            order = np.argsort(-s, axis=1)
            bi = order[:, 0]
            r = np.arange(s.shape[0])
            win_idx[sl] = cidx[sl][r, bi]
            margin[sl] = s[r, bi] - s[r, order[:, 1]]

        # safety net: full-vocab rescore of low-margin tokens
        flagged = np.where(margin < FLAG_THETA)[0]
        if flagged.size:
            s = jnp.einsum("td,vd->tv", jnp.asarray(x[flagged]), jnp.asarray(embed_table))
            win_idx[flagged] = np.asarray(jnp.argmax(s, axis=-1))

    _lap(f"merge+rescore ({flagged.size} flagged)")
    out = embed_table[win_idx].reshape(B, S, D)
    _lap("gather")
    return out
